# revision 12
# baseline (speedup 1.0000x reference)
"""Trainium2 Bass kernel for nn_Encoder_88656714924838 (6-layer dense
transformer encoder with distance-bias attention, d_model=64, 4 heads).

Sharding: pure data parallel - batch 256 split as 32 per core across 8 cores.

v2 design (vs. baseline):
- bf16 residual stream; every PE matmul streams bf16 (1 cycle/row).
- No PE transposes: all layout changes via DMA-transpose XBAR (x/x2/ctx are
  transposed in [128,128] two-batch pairs; V^T -> V per batch).
- Paired stationaries: (Wq0p|Wq1p), (Wq2p|Wq3p), (Wv|Wk) produce two
  projections per stream; full-height [128,512] PSUM evacuations; the
  second half is consumed at PE row-base 64 (hi/lo stationary copies).
- Scores per head at row-base 0/64; exp over 2-batch multi-bank PSUM tiles
  on ACT; softmax denominator via ones-column in V (as baseline).
- LN: per-batch bn_stats/aggr (DVE), batched sqrt (ACT), recip (DVE),
  apply on GPSIMD (SBUF-only engine).
"""

import sys

for _p in ("/opt/trn_rl_repo",):
    if _p not in sys.path:
        sys.path.insert(0, _p)

import numpy as np

D_MODEL = 64
N_HEADS = 4
D_K = 16
D_FF = 512
N_LAYERS = 6
B, L = 256, 128
N_CORES = 8
B_LOC = B // N_CORES
G = 16  # batches per work group
SCALE = 1.0 / np.sqrt(np.float32(D_K))


def _positional_encoding(length=L, d_model=D_MODEL):
    pos = np.arange(length, dtype=np.float32)[:, None]
    div = np.exp(
        np.arange(0, d_model, 2, dtype=np.float32) * (-np.log(10000.0) / d_model)
    )
    pe = np.zeros((length, d_model), dtype=np.float32)
    pe[:, 0::2] = np.sin(pos * div)
    pe[:, 1::2] = np.cos(pos * div)
    return pe


def _split_multi_waits(nc):
    """The walrus build accepts only ONE sync-wait per instruction. Hoist
    extra semaphore waits onto same-engine NoOps just before the carrier."""
    import concourse.mybir as mybir

    k = 0
    for fn in nc.m.functions:
        for blk in fn.blocks:
            new = []
            changed = False
            for inst in blk.instructions:
                si = inst.sync_info
                waits = list(si.on_wait) if (si and si.on_wait) else []
                if len(waits) > 1:
                    changed = True
                    for w in waits[:-1]:
                        k += 1
                        nop = mybir.InstNoOp(name=f"ws-{k}", ins=[], outs=[])
                        nop.engine = inst.engine
                        nop.sync_info = mybir.SyncInfo(on_wait=[w], on_update=[])
                        nc.register_instruction(nop)
                        new.append(nop)
                    si.on_wait = waits[-1:]
                new.append(inst)
            if changed:
                blk.instructions = new


def build_nc(n_layers=N_LAYERS, b_loc=B_LOC, max_phase=5):
    """Build the Bass module. Same program runs on every core (SPMD).
    max_phase: debug aid - 0=P only, 1=+C, 2=+D, 3=+E, 4=+F(W1), 5=all."""
    import concourse.bass as bass
    import concourse.mybir as mybir
    import concourse.tile as tile

    f32 = mybir.dt.float32
    bf16 = mybir.dt.bfloat16
    AF = mybir.ActivationFunctionType
    OP = mybir.AluOpType

    NG = b_loc // G  # number of groups

    nc = bass.Bass("TRN2", target_bir_lowering=False, debug=False)

    # host-prepped layouts (see _host_prep)
    x0b_d = nc.dram_tensor("x0b", [L, b_loc, D_MODEL], f32, kind="ExternalInput")
    x0t2_d = nc.dram_tensor("x0t2", [128, b_loc // 2, 128], bf16, kind="ExternalInput")
    ebt_d = nc.dram_tensor("ebt", [L, b_loc, N_HEADS, L], bf16, kind="ExternalInput")
    s_d = nc.dram_tensor("s", [n_layers, 3, D_MODEL, 128], bf16, kind="ExternalInput")
    wo_d = nc.dram_tensor("wo", [n_layers, D_MODEL, D_MODEL], bf16, kind="ExternalInput")
    w1_d = nc.dram_tensor("w1", [n_layers, D_MODEL, D_FF], bf16, kind="ExternalInput")
    w2_d = nc.dram_tensor("w2", [n_layers, 4, 128, D_MODEL], bf16, kind="ExternalInput")
    out_d = nc.dram_tensor("out", [b_loc, L, D_MODEL], f32, kind="ExternalOutput")

    with tile.TileContext(nc) as tc:
        with (
            tc.tile_pool(name="consts", bufs=1) as consts,
            tc.tile_pool(name="state", bufs=1) as state,
            tc.tile_pool(name="work", bufs=2) as work,
            tc.tile_pool(name="pa", bufs=4, space="PSUM") as pa,
            tc.tile_pool(name="pb", bufs=2, space="PSUM") as pb,
        ):
            eps_t = consts.tile([128, 1], f32)
            nc.vector.memset(eps_t[:], 1e-5)

            # weights: lo copies at partitions 0:64, hi copies at 64:128
            s_lo = consts.tile([64, n_layers, 3, 128], bf16)
            nc.sync.dma_start(out=s_lo[:], in_=s_d.rearrange("n s k m -> k n s m"))
            s_hi = consts.tile([128, n_layers, 3, 128], bf16)
            nc.sync.dma_start(
                out=s_hi[64:128, :, :, :], in_=s_d.rearrange("n s k m -> k n s m")
            )
            wo_lo = consts.tile([64, n_layers, D_MODEL], bf16)
            nc.sync.dma_start(out=wo_lo[:], in_=wo_d.rearrange("n k m -> k n m"))
            w1_lo = consts.tile([64, n_layers, D_FF], bf16)
            nc.sync.dma_start(out=w1_lo[:], in_=w1_d.rearrange("n k m -> k n m"))
            w2_sb = consts.tile([128, n_layers, 4, D_MODEL], bf16)
            nc.sync.dma_start(out=w2_sb[:], in_=w2_d.rearrange("n c k m -> k n c m"))

            # exp(biasT + mask): [128 j, b, h, 128 i]
            eb_sb = state.tile([128, b_loc, N_HEADS, L], bf16)
            nc.sync.dma_start(out=eb_sb[:], in_=ebt_d.ap())

            # residual stream (bf16) + its pair-transposed form, per group
            xbs, xts = [], []
            for gi in range(NG):
                xb = work.tile([128, G, D_MODEL], f32, tag=f"x{gi}", bufs=2)
                nc.sync.dma_start(
                    out=xb[:], in_=x0b_d[:, gi * G:(gi + 1) * G, :]
                )
                xt = work.tile([128, G // 2, 128], bf16, tag=f"xt{gi}", bufs=2)
                nc.sync.dma_start(
                    out=xt[:], in_=x0t2_d[:, gi * (G // 2):(gi + 1) * (G // 2), :]
                )
                xbs.append(xb)
                xts.append(xt)

            for layer in range(n_layers):
                for gi in range(NG):
                    g0 = gi * G
                    x_bf = xbs[gi]
                    xt2 = xts[gi]

                    # ---- P: paired projections ------------------------------
                    # S0=(wq0p|wq1p), S1=(wq2p|wq3p), S2=(wk|wv)
                    # Q [64, h, G, 128] all heads at base 0; KV full height
                    # (kt rows 0:64 for PE, vt rows 64:128 for DMA transpose).
                    Q = work.tile([64, N_HEADS, G, 128], bf16, tag="Q", bufs=1)
                    KV = work.tile([128, G, 128], bf16, tag="KV", bufs=2)
                    ecnt = 0
                    for s in range(3):
                        for par in range(2):
                            lhsT = (
                                s_lo[:, layer, s, :] if par == 0
                                else s_hi[64:128, layer, s, :]
                            )
                            for cc in range(2):
                                pp = pa.tile([128, 512], f32, tag="pp")
                                nc.tensor.matmul(
                                    out=pp[:],
                                    lhsT=lhsT,
                                    rhs=xt2[64 * par:64 * par + 64,
                                            4 * cc:4 * cc + 4, :],
                                    start=True, stop=True,
                                )
                                ppv = pp[:].rearrange("p (m i) -> p m i", m=4)
                                if s < 2:
                                    dtop = Q[:, 2 * s, 8 * cc:8 * cc + 8, :].rearrange(
                                        "p (m two) i -> p two m i", two=2
                                    )[:, par, :, :]
                                    dbot = Q[:, 2 * s + 1, 8 * cc:8 * cc + 8, :].rearrange(
                                        "p (m two) i -> p two m i", two=2
                                    )[:, par, :, :]
                                    if ecnt % 2 == 0:
                                        nc.vector.tensor_copy(out=dtop, in_=ppv[0:64])
                                        nc.scalar.copy(out=dbot, in_=ppv[64:128])
                                    else:
                                        nc.scalar.copy(out=dtop, in_=ppv[0:64])
                                        nc.vector.tensor_copy(out=dbot, in_=ppv[64:128])
                                else:
                                    dkv = KV[:, 8 * cc:8 * cc + 8, :].rearrange(
                                        "p (m two) i -> p two m i", two=2
                                    )[:, par, :, :]
                                    if ecnt % 2 == 0:
                                        nc.vector.tensor_copy(out=dkv, in_=ppv)
                                    else:
                                        nc.scalar.copy(out=dkv, in_=ppv)
                                ecnt += 1
                    # V: dma-transpose vt (KV rows 64:128) -> vp2 [j, 64]
                    vp2 = work.tile([128, G, 64], bf16, tag="vp2", bufs=2)
                    for b in range(G):
                        nc.sync.dma_start(
                            out=vp2[:, b, :], in_=KV[64:128, b, :], transpose=True,
                        )
                    # re-layout with ones column: vp17 [j, b, h, 17] (Pool)
                    vp = work.tile([128, G, N_HEADS, 17], bf16, tag="vp", bufs=2)
                    nc.vector.memset(vp[:, :, :, 16:17], 1.0)
                    nc.gpsimd.tensor_copy(
                        out=vp[:, :, :, 0:16],
                        in_=vp2[:].rearrange("p g (h e) -> p g h e", h=N_HEADS),
                    )

                    if max_phase < 1:
                        continue
                    # ---- C ----
                    at = work.tile([128, G, N_HEADS, 128], bf16, tag="at", bufs=1)
                    for r in range(G // 2):
                        ps_c = pb.tile([128, 1024], f32, tag="pc")
                        psv = ps_c[:].rearrange("p (b h i) -> p b h i", b=2, h=4)
                        for j2 in range(2):
                            b = 2 * r + j2
                            nc.tensor.matmul(
                                out=psv[:, j2, :, :], lhsT=KV[0:64, b, :],
                                rhs=Q[:, :, b, :], start=True, stop=True,
                            )
                        ex = work.tile([128, 2, N_HEADS, 128], bf16, tag="ex", bufs=2)
                        if max_phase == 11:
                            nc.vector.tensor_copy(out=ex[:], in_=psv)
                            continue
                        nc.scalar.activation(out=ex[:], in_=psv, func=AF.Exp)
                        if max_phase == 12:
                            continue
                        nc.vector.tensor_mul(
                            out=at[:, 2 * r:2 * r + 2, :, :],
                            in0=ex[:],
                            in1=eb_sb[:, g0 + 2 * r:g0 + 2 * r + 2, :, :],
                        )

                    if max_phase < 2 or max_phase in (11, 12, 13):
                        continue
                    # ---- D ----
                    ctx_bf = work.tile([128, G, D_MODEL], bf16, tag="ctx", bufs=2)
                    for r in range(G // 4):
                        pd_t = pa.tile([128, 512], f32, tag="pp")
                        pdv = pd_t[:, 0:272].rearrange(
                            "p (b h e) -> p b h e", b=4, h=4
                        )
                        for j4 in range(4):
                            b = 4 * r + j4
                            for h in range(N_HEADS):
                                nc.tensor.matmul(
                                    out=pdv[:, j4, h, :],
                                    lhsT=at[:, b, h, :], rhs=vp[:, b, h, :],
                                    start=True, stop=True,
                                )
                        recip = work.tile([128, 4, N_HEADS, 1], f32, tag="recip",
                                          bufs=4)
                        nc.vector.reciprocal(out=recip[:], in_=pdv[:, :, :, 16:17])
                        nc.vector.tensor_mul(
                            out=ctx_bf[:, 4 * r:4 * r + 4, :].rearrange(
                                "p b (h e) -> p b h e", h=4),
                            in0=pdv[:, :, :, 0:16],
                            in1=recip[:].to_broadcast([128, 4, N_HEADS, 16]),
                        )

                    if max_phase < 3:
                        continue
                    # ---- E ----
                    ctxt2 = work.tile([128, G // 2, 128], bf16, tag="ctxt", bufs=2)
                    for m in range(G // 2):
                        nc.sync.dma_start(
                            out=ctxt2[:, m, :], in_=ctx_bf[:, 2 * m:2 * m + 2, :],
                            transpose=True,
                        )
                    ctxt_lo = work.tile([64, G // 2, 128], bf16, tag="ctxtlo", bufs=2)
                    nc.gpsimd.tensor_copy(out=ctxt_lo[:], in_=ctxt2[64:128, :, :])
                    v1 = work.tile([128, G, D_MODEL], f32, tag="v1", bufs=2)
                    for r in range(G // 4):
                        po = pa.tile([128, 512], f32, tag="pp")
                        pov = po[:, 0:256].rearrange("p (b m) -> p b m", b=4)
                        for j4 in range(4):
                            b = 4 * r + j4
                            par, m = b % 2, b // 2
                            lhsT = (ctxt2[0:64, m, :] if par == 0
                                    else ctxt_lo[:, m, :])
                            nc.tensor.matmul(
                                out=pov[:, j4, :], lhsT=lhsT,
                                rhs=wo_lo[:, layer, :],
                                start=True, stop=True,
                            )
                        nc.vector.tensor_add(
                            out=v1[:, 4 * r:4 * r + 4, :], in0=pov,
                            in1=x_bf[:, 4 * r:4 * r + 4, :],
                        )
                    x2_f = work.tile([128, G, D_MODEL], f32, tag="x2f", bufs=2)
                    _ln(nc, work, eps_t, v1, x2_f, G, mybir)
                    x2_bf = work.tile([128, G, D_MODEL], bf16, tag="x2", bufs=2)
                    nc.gpsimd.tensor_copy(out=x2_bf[:], in_=x2_f[:])
                    x2t2 = work.tile([128, G // 2, 128], bf16, tag="x2t", bufs=2)
                    for m in range(G // 2):
                        nc.scalar.dma_start(
                            out=x2t2[:, m, :], in_=x2_bf[:, 2 * m:2 * m + 2, :],
                            transpose=True,
                        )

                    if max_phase < 4:
                        continue
                    # ---- F ----
                    x2t_lo = work.tile([64, G // 2, 128], bf16, tag="x2tlo", bufs=2)
                    nc.gpsimd.tensor_copy(out=x2t_lo[:], in_=x2t2[64:128, :, :])
                    ht = work.tile([128, 4, G, 128], bf16, tag="ht", bufs=1)
                    rcnt = 0
                    for c in range(4):
                        for cc in range(2):
                            pw = pb.tile([128, 1024], f32, tag="pc")
                            for par in range(2):
                                rhs = (
                                    x2t2[0:64, 4 * cc:4 * cc + 4, :] if par == 0
                                    else x2t_lo[:, 4 * cc:4 * cc + 4, :]
                                )
                                nc.tensor.matmul(
                                    out=pw[:, 512 * par:512 * par + 512],
                                    lhsT=w1_lo[:, layer, 128 * c:128 * c + 128],
                                    rhs=rhs,
                                    start=True, stop=True,
                                )
                            src_ap = pw[:].rearrange(
                                "p (two m i) -> p two m i", two=2, m=4)
                            dst = ht[:, c, 8 * cc:8 * cc + 8, :].rearrange(
                                "p (m two) i -> p two m i", two=2)
                            if rcnt % 2 == 0:
                                nc.scalar.activation(out=dst, in_=src_ap, func=AF.Relu)
                            else:
                                nc.vector.tensor_scalar_max(dst, src_ap, 0.0)
                            rcnt += 1
                    if max_phase < 5:
                        continue
                    v2 = work.tile([128, G, D_MODEL], f32, tag="v2", bufs=2)
                    for r in range(G // 4):
                        py = pa.tile([128, 512], f32, tag="pp")
                        pyv = py[:, 0:256].rearrange("p (b m) -> p b m", b=4)
                        for j4 in range(4):
                            b = 4 * r + j4
                            for c in range(4):
                                nc.tensor.matmul(
                                    out=pyv[:, j4, :],
                                    lhsT=ht[:, c, b, :],
                                    rhs=w2_sb[:, layer, c, :],
                                    start=(c == 0), stop=(c == 3),
                                )
                        nc.vector.tensor_add(
                            out=v2[:, 4 * r:4 * r + 4, :], in0=pyv,
                            in1=x2_f[:, 4 * r:4 * r + 4, :],
                        )
                    if layer < n_layers - 1:
                        xb_n = work.tile([128, G, D_MODEL], f32, tag=f"x{gi}",
                                         bufs=2)
                        _ln(nc, work, eps_t, v2, xb_n, G, mybir)
                        xsh = work.tile([128, G, D_MODEL], bf16, tag="xsh", bufs=2)
                        nc.gpsimd.tensor_copy(out=xsh[:], in_=xb_n[:])
                        xt_n = work.tile([128, G // 2, 128], bf16, tag=f"xt{gi}",
                                         bufs=2)
                        for m in range(G // 2):
                            nc.scalar.dma_start(
                                out=xt_n[:, m, :], in_=xsh[:, 2 * m:2 * m + 2, :],
                                transpose=True,
                            )
                        xbs[gi] = xb_n
                        xts[gi] = xt_n
                    else:
                        xout = work.tile([128, G, D_MODEL], f32, tag="xout",
                                         bufs=1)
                        _ln(nc, work, eps_t, v2, xout, G, mybir)
                        nc.sync.dma_start(
                            out=out_d[g0:g0 + G].rearrange("b l d -> l b d"),
                            in_=xout[:],
                        )

            if max_phase < 5:
                for gi in range(NG):
                    xo = work.tile([128, G, D_MODEL], f32, tag="xout", bufs=1)
                    nc.vector.tensor_copy(out=xo[:], in_=xbs[gi][:])
                    nc.sync.dma_start(
                        out=out_d[gi * G:gi * G + G].rearrange("b l d -> l b d"),
                        in_=xo[:],
                    )

    _split_multi_waits(nc)
    return nc


def _ln(nc, work, eps_t, v, out_t, G, mybir):
    """LayerNorm over free dim 64: per-batch bn_stats/aggr (DVE), batched
    sqrt (ACT), recip (DVE), per-batch apply on GPSIMD."""
    f32 = mybir.dt.float32
    OP = mybir.AluOpType
    st6 = work.tile([128, G, 6], f32, tag="st6", bufs=2)
    mv = work.tile([128, G, 2], f32, tag="mv", bufs=2)
    for b in range(G):
        nc.vector.bn_stats(out=st6[:, b, :], in_=v[:, b, :])
        nc.vector.bn_aggr(out=mv[:, b, :], in_=st6[:, b, :])
    std = work.tile([128, G, 1], f32, tag="std", bufs=2)
    nc.scalar.activation(
        out=std[:], in_=mv[:, :, 1:2],
        func=mybir.ActivationFunctionType.Sqrt, bias=eps_t[:, 0:1], scale=1.0,
    )
    rstd = work.tile([128, G, 1], f32, tag="rstd", bufs=2)
    nc.vector.reciprocal(out=rstd[:], in_=std[:])
    for b in range(G):
        nc.gpsimd.tensor_scalar(
            out=out_t[:, b, :], in0=v[:, b, :],
            scalar1=mv[:, b, 0:1], scalar2=rstd[:, b, 0:1],
            op0=OP.subtract, op1=OP.mult,
        )


def _host_prep(inputs):
    import ml_dtypes

    bf16 = ml_dtypes.bfloat16

    enc = np.asarray(inputs["enc_inputs"])
    deg = np.asarray(inputs["degree_s"])
    MD = np.asarray(inputs["MD"])
    src_emb = np.asarray(inputs["src_emb"], dtype=np.float32)
    deg_emb = np.asarray(inputs["deg_emb"], dtype=np.float32)
    md_emb = np.asarray(inputs["md_emb"], dtype=np.float32)

    x0 = (src_emb[enc] + deg_emb[deg] + _positional_encoding()[None]).astype(
        np.float32
    )
    x0_bf = x0.astype(bf16)
    # [l, b, d] f32 layout for the residual stream
    x0_lbd = np.ascontiguousarray(x0.transpose(1, 0, 2))
    # pair-transposed: [128 rows=(2 x 64 dims), pair m, 128 cols=L]
    x0t = x0_bf.transpose(0, 2, 1)  # [B, 64, 128]
    x0t2 = np.ascontiguousarray(
        x0t.reshape(B // 2, 2 * D_MODEL, L).transpose(1, 0, 2)
    )  # [128, B/2, 128]

    # bias[b,i,j,h] -> scores^T layout [j, b, h, i]; fold pad mask; exp
    bias_t = md_emb[MD].transpose(2, 0, 3, 1)  # [j, b, h, i]
    mask = np.where(enc == 0, np.float32(-1e9), np.float32(0.0))  # keys
    with np.errstate(under="ignore"):
        ebt = np.exp(bias_t + mask.T[:, :, None, None], dtype=np.float32)
    ebt = np.ascontiguousarray(ebt).astype(bf16)

    wq = np.asarray(inputs["Wq"], dtype=np.float32) * SCALE
    wk = np.asarray(inputs["Wk"], dtype=np.float32)
    wv = np.asarray(inputs["Wv"], dtype=np.float32)

    def padhead(h):
        out = np.zeros((N_LAYERS, D_MODEL, D_MODEL), dtype=np.float32)
        sl = slice(D_K * h, D_K * (h + 1))
        out[:, :, sl] = wq[:, :, sl]
        return out

    S = np.zeros((N_LAYERS, 3, D_MODEL, 128), dtype=np.float32)
    S[:, 0, :, 0:64] = padhead(0)
    S[:, 0, :, 64:128] = padhead(1)
    S[:, 1, :, 0:64] = padhead(2)
    S[:, 1, :, 64:128] = padhead(3)
    S[:, 2, :, 0:64] = wk
    S[:, 2, :, 64:128] = wv
    S = S.astype(bf16)

    wo = np.asarray(inputs["Wo"], dtype=np.float32).astype(bf16)
    w1 = np.asarray(inputs["W1"], dtype=np.float32).astype(bf16)
    w2 = np.ascontiguousarray(
        np.asarray(inputs["W2"], dtype=np.float32).reshape(N_LAYERS, 4, 128, D_MODEL)
    ).astype(bf16)
    return x0_lbd, x0t2, ebt, S, wo, w1, w2


_NC_CACHE = {}


def run(inputs, trace=False, **spmd_kwargs):
    """Run on the 8 cores; returns (full_output, BassKernelResults)."""
    from concourse.bass_utils import run_bass_kernel_spmd

    x0_lbd, x0t2, ebt, S, wo, w1, w2 = _host_prep(inputs)

    if "nc" not in _NC_CACHE:
        _NC_CACHE["nc"] = build_nc()
    nc = _NC_CACHE["nc"]

    in_maps = []
    for c in range(N_CORES):
        sl = slice(c * B_LOC, (c + 1) * B_LOC)
        sl2 = slice(c * (B_LOC // 2), (c + 1) * (B_LOC // 2))
        in_maps.append(
            dict(
                x0b=np.ascontiguousarray(x0_lbd[:, sl, :]),
                x0t2=np.ascontiguousarray(x0t2[:, sl2, :]),
                ebt=np.ascontiguousarray(ebt[:, sl, :, :]),
                s=S, wo=wo, w1=w1, w2=w2,
            )
        )

    res = run_bass_kernel_spmd(
        nc, in_maps, core_ids=list(range(N_CORES)), trace=trace, **spmd_kwargs
    )
    out = np.concatenate([res.results[c]["out"] for c in range(N_CORES)], axis=0)
    return out.astype(np.float32), res


def kernel(**inputs):
    out, _ = run(inputs)
    return out


def _jit_single_core(nc):
    """Build a single-device jitted callable for nc (same program as SPMD)."""
    import jax
    from concourse import bass2jax
    from concourse import mybir

    bass2jax.install_neuronx_cc_hook()
    in_names, out_names, out_avals, zero_outs = [], [], [], []
    partition_name = nc.partition_id_tensor.name if nc.partition_id_tensor else None
    for alloc in nc.m.functions[0].allocations:
        if not isinstance(alloc, mybir.MemoryLocationSet):
            continue
        name = alloc.memorylocations[0].name
        if alloc.kind == "ExternalInput":
            if name != partition_name:
                in_names.append(name)
        elif alloc.kind == "ExternalOutput":
            out_names.append(name)
            shape = tuple(alloc.tensor_shape)
            dtype = mybir.dt.np(alloc.dtype)
            out_avals.append(jax.core.ShapedArray(shape, dtype))
            zero_outs.append(np.zeros(shape, dtype))
    n_params = len(in_names)
    all_names = in_names + out_names + ([partition_name] if partition_name else [])
    donate = tuple(range(n_params, n_params + len(out_names)))

    def _body(*args):
        operands = list(args)
        if partition_name is not None:
            operands.append(bass2jax.partition_id_tensor())
        outs = bass2jax._bass_exec_p.bind(
            *operands,
            out_avals=tuple(out_avals),
            in_names=tuple(all_names),
            out_names=tuple(out_names),
            lowering_input_output_aliases=(),
            sim_require_finite=True,
            sim_require_nnan=True,
            nc=nc,
        )
        return tuple(outs)

    jfn = jax.jit(_body, donate_argnums=donate, keep_unused=True)
    return jfn, in_names, zero_outs


def bench_marginal(inputs, iters=24, reps=2):
    """Per-execution device time via async dispatch pipelining."""
    import time

    import jax

    x0_lbd, x0t2, ebt, S, wo, w1, w2 = _host_prep(inputs)
    if "nc" not in _NC_CACHE:
        _NC_CACHE["nc"] = build_nc()
    nc = _NC_CACHE["nc"]
    in_map = dict(
        x0b=np.ascontiguousarray(x0_lbd[:, :B_LOC, :]),
        x0t2=np.ascontiguousarray(x0t2[:, :B_LOC // 2, :]),
        ebt=np.ascontiguousarray(ebt[:, :B_LOC, :, :]),
        s=S, wo=wo, w1=w1, w2=w2,
    )
    jfn, in_names, zero_outs = _jit_single_core(nc)
    dev = jax.devices()[0]
    ins_dev = [jax.device_put(np.asarray(in_map[n]), dev) for n in in_names]
    n_zsets = (iters + 2) * reps + 4
    zsets = [
        [jax.device_put(z.copy(), dev) for z in zero_outs] for _ in range(n_zsets)
    ]
    jax.block_until_ready(zsets)
    jax.block_until_ready(ins_dev)
    state = {"zi": 0}

    def run_m(m):
        outs = []
        t0 = time.perf_counter()
        for _ in range(m):
            outs.append(jfn(*ins_dev, *zsets[state["zi"]]))
            state["zi"] += 1
        jax.block_until_ready(outs)
        return time.perf_counter() - t0

    run_m(1)  # warm (compiles)
    t1s, tns = [], []
    for _ in range(reps):
        t1s.append(run_m(1))
        tns.append(run_m(iters))
    marginal_ns = (min(tns) - min(t1s)) / (iters - 1) * 1e9
    return dict(
        est_exec_ns=marginal_ns,
        t1_ns=min(t1s) * 1e9,
        tn_ns=min(tns) * 1e9,
        t1s=t1s,
        tns=tns,
        iters=iters,
    )


if __name__ == "__main__":
    print("kernel module ok")


# revision 19
# speedup vs baseline: 1.7383x; 1.7383x over previous
"""Trainium2 Bass kernel for nn_Encoder_88656714924838 (6-layer dense
transformer encoder with distance-bias attention, d_model=64, 4 heads).

Sharding: pure data parallel - batch 256 split as 32 per core across 8 cores.

v2 design (vs. baseline):
- bf16 residual stream; every PE matmul streams bf16 (1 cycle/row).
- No PE transposes: all layout changes via DMA-transpose XBAR (x/x2/ctx are
  transposed in [128,128] two-batch pairs; V^T -> V per batch).
- Paired stationaries: (Wq0p|Wq1p), (Wq2p|Wq3p), (Wv|Wk) produce two
  projections per stream; full-height [128,512] PSUM evacuations; the
  second half is consumed at PE row-base 64 (hi/lo stationary copies).
- Scores per head at row-base 0/64; exp over 2-batch multi-bank PSUM tiles
  on ACT; softmax denominator via ones-column in V (as baseline).
- LN: per-batch bn_stats/aggr (DVE), batched sqrt (ACT), recip (DVE),
  apply on GPSIMD (SBUF-only engine).
"""

import sys

for _p in ("/opt/trn_rl_repo",):
    if _p not in sys.path:
        sys.path.insert(0, _p)

import numpy as np

D_MODEL = 64
N_HEADS = 4
D_K = 16
D_FF = 512
N_LAYERS = 6
B, L = 256, 128
N_CORES = 8
B_LOC = B // N_CORES
G = 16  # batches per work group
SCALE = 1.0 / np.sqrt(np.float32(D_K))


def _positional_encoding(length=L, d_model=D_MODEL):
    pos = np.arange(length, dtype=np.float32)[:, None]
    div = np.exp(
        np.arange(0, d_model, 2, dtype=np.float32) * (-np.log(10000.0) / d_model)
    )
    pe = np.zeros((length, d_model), dtype=np.float32)
    pe[:, 0::2] = np.sin(pos * div)
    pe[:, 1::2] = np.cos(pos * div)
    return pe


def _split_multi_waits(nc):
    """The walrus build accepts only ONE sync-wait per instruction. Hoist
    extra semaphore waits onto same-engine NoOps just before the carrier."""
    import concourse.mybir as mybir

    k = 0
    for fn in nc.m.functions:
        for blk in fn.blocks:
            new = []
            changed = False
            for inst in blk.instructions:
                si = inst.sync_info
                waits = list(si.on_wait) if (si and si.on_wait) else []
                if len(waits) > 1:
                    changed = True
                    for w in waits[:-1]:
                        k += 1
                        nop = mybir.InstNoOp(name=f"ws-{k}", ins=[], outs=[])
                        nop.engine = inst.engine
                        nop.sync_info = mybir.SyncInfo(on_wait=[w], on_update=[])
                        nc.register_instruction(nop)
                        new.append(nop)
                    si.on_wait = waits[-1:]
                new.append(inst)
            if changed:
                blk.instructions = new


def build_nc(n_layers=N_LAYERS, b_loc=B_LOC, max_phase=5):
    """Build the Bass module. Same program runs on every core (SPMD).
    max_phase: debug aid - 0=P only, 1=+C, 2=+D, 3=+E, 4=+F(W1), 5=all."""
    import concourse.bass as bass
    import concourse.mybir as mybir
    import concourse.tile as tile

    f32 = mybir.dt.float32
    bf16 = mybir.dt.bfloat16
    AF = mybir.ActivationFunctionType
    OP = mybir.AluOpType

    NG = b_loc // G  # number of groups
    no_dma = max_phase in (31, 33)
    no_pool = max_phase in (32, 33)

    nc = bass.Bass("TRN2", target_bir_lowering=False, debug=False)

    # host-prepped layouts (see _host_prep)
    x0b_d = nc.dram_tensor("x0b", [L, b_loc, D_MODEL], f32, kind="ExternalInput")
    x0t2_d = nc.dram_tensor("x0t2", [128, b_loc // 2, 128], bf16, kind="ExternalInput")
    ebt_d = nc.dram_tensor("ebt", [L, b_loc, N_HEADS, L], bf16, kind="ExternalInput")
    s_d = nc.dram_tensor("s", [n_layers, 3, D_MODEL, 128], bf16, kind="ExternalInput")
    wo_d = nc.dram_tensor("wo", [n_layers, D_MODEL, D_MODEL], bf16, kind="ExternalInput")
    w1_d = nc.dram_tensor("w1", [n_layers, D_MODEL, D_FF], bf16, kind="ExternalInput")
    w2_d = nc.dram_tensor("w2", [n_layers, 4, 128, D_MODEL], bf16, kind="ExternalInput")
    out_d = nc.dram_tensor("out", [b_loc, L, D_MODEL], f32, kind="ExternalOutput")

    with tile.TileContext(nc) as tc:
        with (
            tc.tile_pool(name="consts", bufs=1) as consts,
            tc.tile_pool(name="state", bufs=1) as state,
            tc.tile_pool(name="work", bufs=2) as work,
            tc.tile_pool(name="pa", bufs=4, space="PSUM") as pa,
            tc.tile_pool(name="pb", bufs=2, space="PSUM") as pb,
        ):
            eps_t = consts.tile([128, 1], f32)
            nc.vector.memset(eps_t[:], 1e-5)

            # weights: lo copies at partitions 0:64, hi copies at 64:128
            s_lo = consts.tile([64, n_layers, 3, 128], bf16)
            nc.sync.dma_start(out=s_lo[:], in_=s_d.rearrange("n s k m -> k n s m"))
            s_hi = consts.tile([128, n_layers, 3, 128], bf16)
            nc.sync.dma_start(
                out=s_hi[64:128, :, :, :], in_=s_d.rearrange("n s k m -> k n s m")
            )
            wo_lo = consts.tile([64, n_layers, D_MODEL], bf16)
            nc.sync.dma_start(out=wo_lo[:], in_=wo_d.rearrange("n k m -> k n m"))
            w1_lo = consts.tile([64, n_layers, D_FF], bf16)
            nc.sync.dma_start(out=w1_lo[:], in_=w1_d.rearrange("n k m -> k n m"))
            w2_sb = consts.tile([128, n_layers, 4, D_MODEL], bf16)
            nc.sync.dma_start(out=w2_sb[:], in_=w2_d.rearrange("n c k m -> k n c m"))

            # exp(biasT + mask): [128 j, b, h, 128 i]
            eb_sb = state.tile([128, b_loc, N_HEADS, L], bf16)
            nc.sync.dma_start(out=eb_sb[:], in_=ebt_d.ap())

            # residual stream (bf16) + its pair-transposed form, per group
            xbs, xts = [], []
            for gi in range(NG):
                xb = work.tile([128, G, D_MODEL], f32, tag=f"x{gi}", bufs=2)
                nc.sync.dma_start(
                    out=xb[:], in_=x0b_d[:, gi * G:(gi + 1) * G, :]
                )
                xt = work.tile([128, G // 2, 128], bf16, tag=f"xt{gi}", bufs=2)
                nc.sync.dma_start(
                    out=xt[:], in_=x0t2_d[:, gi * (G // 2):(gi + 1) * (G // 2), :]
                )
                xbs.append(xb)
                xts.append(xt)

            for layer in range(n_layers):
                for gi in range(NG):
                    g0 = gi * G
                    x_bf = xbs[gi]
                    xt2 = xts[gi]

                    # ---- P: paired projections ------------------------------
                    # S0=(wq0p|wq1p), S1=(wq2p|wq3p), S2=(wk|wv)
                    # Q [64, h, G, 128] all heads at base 0; KV full height
                    # (kt rows 0:64 for PE, vt rows 64:128 for DMA transpose).
                    Q = work.tile([64, N_HEADS, G, 128], bf16, tag="Q", bufs=1)
                    KV = work.tile([128, G, 128], bf16, tag="KV", bufs=2)
                    ecnt = 0
                    for s in range(3):
                        for par in range(2):
                            lhsT = (
                                s_lo[:, layer, s, :] if par == 0
                                else s_hi[64:128, layer, s, :]
                            )
                            for cc in range(2):
                                pp = pa.tile([128, 512], f32, tag="pp")
                                nc.tensor.matmul(
                                    out=pp[:],
                                    lhsT=lhsT,
                                    rhs=xt2[64 * par:64 * par + 64,
                                            4 * cc:4 * cc + 4, :],
                                    start=True, stop=True,
                                )
                                ppv = pp[:].rearrange("p (m i) -> p m i", m=4)
                                if s < 2:
                                    dtop = Q[:, 2 * s, 8 * cc:8 * cc + 8, :].rearrange(
                                        "p (m two) i -> p two m i", two=2
                                    )[:, par, :, :]
                                    dbot = Q[:, 2 * s + 1, 8 * cc:8 * cc + 8, :].rearrange(
                                        "p (m two) i -> p two m i", two=2
                                    )[:, par, :, :]
                                    if ecnt % 2 == 0:
                                        nc.vector.tensor_copy(out=dtop, in_=ppv[0:64])
                                        nc.scalar.copy(out=dbot, in_=ppv[64:128])
                                    else:
                                        nc.scalar.copy(out=dtop, in_=ppv[0:64])
                                        nc.vector.tensor_copy(out=dbot, in_=ppv[64:128])
                                else:
                                    dkv = KV[:, 8 * cc:8 * cc + 8, :].rearrange(
                                        "p (m two) i -> p two m i", two=2
                                    )[:, par, :, :]
                                    if ecnt % 2 == 0:
                                        nc.vector.tensor_copy(out=dkv, in_=ppv)
                                    else:
                                        nc.scalar.copy(out=dkv, in_=ppv)
                                ecnt += 1
                    # V: one blocked dma-transpose (KV rows 64:128) -> vp2
                    vp2 = work.tile([128, G, 64], bf16, tag="vp2", bufs=2)
                    nc.sync.dma_start(
                        out=vp2[:], in_=KV[64:128, :, :], transpose=True,
                    )
                    # re-layout with ones column: vp17 [j, b, h, 17] (DVE 4x)
                    vp = work.tile([128, G, N_HEADS, 17], bf16, tag="vp", bufs=2)
                    nc.vector.memset(vp[:, :, :, 16:17], 1.0)
                    nc.vector.tensor_copy(
                        out=vp[:, :, :, 0:16],
                        in_=vp2[:].rearrange("p g (h e) -> p g h e", h=N_HEADS),
                    )

                    if max_phase < 1 or max_phase == 21:
                        continue
                    # ---- C ----
                    at = work.tile([128, G, N_HEADS, 128], bf16, tag="at", bufs=1)
                    for r in range(G // 2):
                        ps_c = pb.tile([128, 1024], f32, tag="pc")
                        psv = ps_c[:].rearrange("p (b h i) -> p b h i", b=2, h=4)
                        for j2 in range(2):
                            b = 2 * r + j2
                            nc.tensor.matmul(
                                out=psv[:, j2, :, :], lhsT=KV[0:64, b, :],
                                rhs=Q[:, :, b, :], start=True, stop=True,
                            )
                        ex = work.tile([128, 2, N_HEADS, 128], bf16, tag="ex", bufs=2)
                        if max_phase == 11:
                            nc.vector.tensor_copy(out=ex[:], in_=psv)
                            continue
                        nc.scalar.activation(out=ex[:], in_=psv, func=AF.Exp)
                        if max_phase == 12:
                            continue
                        nc.vector.tensor_mul(
                            out=at[:, 2 * r:2 * r + 2, :, :],
                            in0=ex[:],
                            in1=eb_sb[:, g0 + 2 * r:g0 + 2 * r + 2, :, :],
                        )

                    if max_phase < 2 or max_phase in (11, 12, 13):
                        continue
                    # ---- D ----
                    ctx_bf = work.tile([128, G, D_MODEL], bf16, tag="ctx", bufs=2)
                    for r in range(G // 4):
                        pd_t = pa.tile([128, 512], f32, tag="pp")
                        pdv = pd_t[:, 0:272].rearrange(
                            "p (b h e) -> p b h e", b=4, h=4
                        )
                        for j4 in range(4):
                            b = 4 * r + j4
                            for h in range(N_HEADS):
                                nc.tensor.matmul(
                                    out=pdv[:, j4, h, :],
                                    lhsT=at[:, b, h, :], rhs=vp[:, b, h, :],
                                    start=True, stop=True,
                                )
                        recip = work.tile([128, 4, N_HEADS, 1], f32, tag="recip",
                                          bufs=4)
                        nc.vector.reciprocal(out=recip[:], in_=pdv[:, :, :, 16:17])
                        nc.vector.tensor_mul(
                            out=ctx_bf[:, 4 * r:4 * r + 4, :].rearrange(
                                "p b (h e) -> p b h e", h=4),
                            in0=pdv[:, :, :, 0:16],
                            in1=recip[:].to_broadcast([128, 4, N_HEADS, 16]),
                        )

                    if max_phase < 3:
                        continue
                    # ---- E ----
                    ctxt2 = work.tile([128, G // 2, 128], bf16, tag="ctxt", bufs=2)
                    nc.sync.dma_start(out=ctxt2[:], in_=ctx_bf[:], transpose=True)
                    ctxt_lo = work.tile([64, G // 2, 128], bf16, tag="ctxtlo", bufs=2)
                    nc.vector.tensor_copy(out=ctxt_lo[:], in_=ctxt2[64:128, :, :])
                    v1 = work.tile([128, G, D_MODEL], f32, tag="v1", bufs=2)
                    for r in range(G // 4):
                        po = pa.tile([128, 512], f32, tag="pp")
                        pov = po[:, 0:256].rearrange("p (b m) -> p b m", b=4)
                        for j4 in range(4):
                            b = 4 * r + j4
                            par, m = b % 2, b // 2
                            lhsT = (ctxt2[0:64, m, :] if par == 0
                                    else ctxt_lo[:, m, :])
                            nc.tensor.matmul(
                                out=pov[:, j4, :], lhsT=lhsT,
                                rhs=wo_lo[:, layer, :],
                                start=True, stop=True,
                            )
                        nc.vector.tensor_add(
                            out=v1[:, 4 * r:4 * r + 4, :], in0=pov,
                            in1=x_bf[:, 4 * r:4 * r + 4, :],
                        )
                    x2_f = work.tile([128, G, D_MODEL], f32, tag="x2f", bufs=2)
                    _ln(nc, work, eps_t, v1, x2_f, G, mybir)
                    x2_bf = work.tile([128, G, D_MODEL], bf16, tag="x2", bufs=2)
                    nc.vector.tensor_copy(out=x2_bf[:], in_=x2_f[:])
                    x2t2 = work.tile([128, G // 2, 128], bf16, tag="x2t", bufs=2)
                    nc.scalar.dma_start(out=x2t2[:], in_=x2_bf[:], transpose=True)

                    if max_phase < 4:
                        continue
                    # ---- F ----
                    x2t_lo = work.tile([64, G // 2, 128], bf16, tag="x2tlo", bufs=2)
                    nc.vector.tensor_copy(out=x2t_lo[:], in_=x2t2[64:128, :, :])
                    ht = work.tile([128, 4, G, 128], bf16, tag="ht", bufs=1)
                    rcnt = 0
                    for c in range(4):
                        for cc in range(2):
                            pw = pb.tile([128, 1024], f32, tag="pc")
                            for par in range(2):
                                rhs = (
                                    x2t2[0:64, 4 * cc:4 * cc + 4, :] if par == 0
                                    else x2t_lo[:, 4 * cc:4 * cc + 4, :]
                                )
                                nc.tensor.matmul(
                                    out=pw[:, 512 * par:512 * par + 512],
                                    lhsT=w1_lo[:, layer, 128 * c:128 * c + 128],
                                    rhs=rhs,
                                    start=True, stop=True,
                                )
                            src_ap = pw[:].rearrange(
                                "p (two m i) -> p two m i", two=2, m=4)
                            dst = ht[:, c, 8 * cc:8 * cc + 8, :].rearrange(
                                "p (m two) i -> p two m i", two=2)
                            if rcnt % 2 == 0:
                                nc.scalar.activation(out=dst, in_=src_ap, func=AF.Relu)
                            else:
                                nc.vector.tensor_scalar_max(dst, src_ap, 0.0)
                            rcnt += 1
                    if max_phase < 5:
                        continue
                    v2 = work.tile([128, G, D_MODEL], f32, tag="v2", bufs=2)
                    for r in range(G // 4):
                        py = pa.tile([128, 512], f32, tag="pp")
                        pyv = py[:, 0:256].rearrange("p (b m) -> p b m", b=4)
                        for j4 in range(4):
                            b = 4 * r + j4
                            for c in range(4):
                                nc.tensor.matmul(
                                    out=pyv[:, j4, :],
                                    lhsT=ht[:, c, b, :],
                                    rhs=w2_sb[:, layer, c, :],
                                    start=(c == 0), stop=(c == 3),
                                )
                        nc.vector.tensor_add(
                            out=v2[:, 4 * r:4 * r + 4, :], in0=pyv,
                            in1=x2_f[:, 4 * r:4 * r + 4, :],
                        )
                    if layer < n_layers - 1:
                        xb_n = work.tile([128, G, D_MODEL], f32, tag=f"x{gi}",
                                         bufs=2)
                        _ln(nc, work, eps_t, v2, xb_n, G, mybir)
                        xsh = work.tile([128, G, D_MODEL], bf16, tag="xsh", bufs=2)
                        nc.vector.tensor_copy(out=xsh[:], in_=xb_n[:])
                        xt_n = work.tile([128, G // 2, 128], bf16, tag=f"xt{gi}",
                                         bufs=2)
                        nc.scalar.dma_start(out=xt_n[:], in_=xsh[:], transpose=True)
                        xbs[gi] = xb_n
                        xts[gi] = xt_n
                    else:
                        xout = work.tile([128, G, D_MODEL], f32, tag="xout",
                                         bufs=1)
                        _ln(nc, work, eps_t, v2, xout, G, mybir)
                        nc.sync.dma_start(
                            out=out_d[g0:g0 + G].rearrange("b l d -> l b d"),
                            in_=xout[:],
                        )

            if max_phase < 5:
                for gi in range(NG):
                    xo = work.tile([128, G, D_MODEL], f32, tag="xout", bufs=1)
                    nc.vector.tensor_copy(out=xo[:], in_=xbs[gi][:])
                    nc.sync.dma_start(
                        out=out_d[gi * G:gi * G + G].rearrange("b l d -> l b d"),
                        in_=xo[:],
                    )

    _split_multi_waits(nc)
    return nc


def _ln(nc, work, eps_t, v, out_t, G, mybir, apply_eng=None):
    """LayerNorm over free dim 64: per-batch bn_stats/aggr (DVE), batched
    sqrt (ACT), recip (DVE), per-batch apply on GPSIMD."""
    f32 = mybir.dt.float32
    OP = mybir.AluOpType
    st6 = work.tile([128, G, 6], f32, tag="st6", bufs=2)
    mv = work.tile([128, G, 2], f32, tag="mv", bufs=2)
    for b in range(G):
        nc.vector.bn_stats(out=st6[:, b, :], in_=v[:, b, :])
        nc.vector.bn_aggr(out=mv[:, b, :], in_=st6[:, b, :])
    std = work.tile([128, G, 1], f32, tag="std", bufs=2)
    nc.scalar.activation(
        out=std[:], in_=mv[:, :, 1:2],
        func=mybir.ActivationFunctionType.Sqrt, bias=eps_t[:, 0:1], scale=1.0,
    )
    rstd = work.tile([128, G, 1], f32, tag="rstd", bufs=2)
    nc.vector.reciprocal(out=rstd[:], in_=std[:])
    if apply_eng is None:
        apply_eng = nc.vector
    for b in range(G):
        apply_eng.tensor_scalar(
            out=out_t[:, b, :], in0=v[:, b, :],
            scalar1=mv[:, b, 0:1], scalar2=rstd[:, b, 0:1],
            op0=OP.subtract, op1=OP.mult,
        )


def _host_prep(inputs):
    import ml_dtypes

    bf16 = ml_dtypes.bfloat16

    enc = np.asarray(inputs["enc_inputs"])
    deg = np.asarray(inputs["degree_s"])
    MD = np.asarray(inputs["MD"])
    src_emb = np.asarray(inputs["src_emb"], dtype=np.float32)
    deg_emb = np.asarray(inputs["deg_emb"], dtype=np.float32)
    md_emb = np.asarray(inputs["md_emb"], dtype=np.float32)

    x0 = (src_emb[enc] + deg_emb[deg] + _positional_encoding()[None]).astype(
        np.float32
    )
    x0_bf = x0.astype(bf16)
    # [l, b, d] f32 layout for the residual stream
    x0_lbd = np.ascontiguousarray(x0.transpose(1, 0, 2))
    # pair-transposed: [128 rows=(2 x 64 dims), pair m, 128 cols=L]
    x0t = x0_bf.transpose(0, 2, 1)  # [B, 64, 128]
    x0t2 = np.ascontiguousarray(
        x0t.reshape(B // 2, 2 * D_MODEL, L).transpose(1, 0, 2)
    )  # [128, B/2, 128]

    # bias[b,i,j,h] -> scores^T layout [j, b, h, i]; fold pad mask; exp
    bias_t = md_emb[MD].transpose(2, 0, 3, 1)  # [j, b, h, i]
    mask = np.where(enc == 0, np.float32(-1e9), np.float32(0.0))  # keys
    with np.errstate(under="ignore"):
        ebt = np.exp(bias_t + mask.T[:, :, None, None], dtype=np.float32)
    ebt = np.ascontiguousarray(ebt).astype(bf16)

    wq = np.asarray(inputs["Wq"], dtype=np.float32) * SCALE
    wk = np.asarray(inputs["Wk"], dtype=np.float32)
    wv = np.asarray(inputs["Wv"], dtype=np.float32)

    def padhead(h):
        out = np.zeros((N_LAYERS, D_MODEL, D_MODEL), dtype=np.float32)
        sl = slice(D_K * h, D_K * (h + 1))
        out[:, :, sl] = wq[:, :, sl]
        return out

    S = np.zeros((N_LAYERS, 3, D_MODEL, 128), dtype=np.float32)
    S[:, 0, :, 0:64] = padhead(0)
    S[:, 0, :, 64:128] = padhead(1)
    S[:, 1, :, 0:64] = padhead(2)
    S[:, 1, :, 64:128] = padhead(3)
    S[:, 2, :, 0:64] = wk
    S[:, 2, :, 64:128] = wv
    S = S.astype(bf16)

    wo = np.asarray(inputs["Wo"], dtype=np.float32).astype(bf16)
    w1 = np.asarray(inputs["W1"], dtype=np.float32).astype(bf16)
    w2 = np.ascontiguousarray(
        np.asarray(inputs["W2"], dtype=np.float32).reshape(N_LAYERS, 4, 128, D_MODEL)
    ).astype(bf16)
    return x0_lbd, x0t2, ebt, S, wo, w1, w2


_NC_CACHE = {}


def run(inputs, trace=False, **spmd_kwargs):
    """Run on the 8 cores; returns (full_output, BassKernelResults)."""
    from concourse.bass_utils import run_bass_kernel_spmd

    x0_lbd, x0t2, ebt, S, wo, w1, w2 = _host_prep(inputs)

    if "nc" not in _NC_CACHE:
        _NC_CACHE["nc"] = build_nc()
    nc = _NC_CACHE["nc"]

    in_maps = []
    for c in range(N_CORES):
        sl = slice(c * B_LOC, (c + 1) * B_LOC)
        sl2 = slice(c * (B_LOC // 2), (c + 1) * (B_LOC // 2))
        in_maps.append(
            dict(
                x0b=np.ascontiguousarray(x0_lbd[:, sl, :]),
                x0t2=np.ascontiguousarray(x0t2[:, sl2, :]),
                ebt=np.ascontiguousarray(ebt[:, sl, :, :]),
                s=S, wo=wo, w1=w1, w2=w2,
            )
        )

    res = run_bass_kernel_spmd(
        nc, in_maps, core_ids=list(range(N_CORES)), trace=trace, **spmd_kwargs
    )
    out = np.concatenate([res.results[c]["out"] for c in range(N_CORES)], axis=0)
    return out.astype(np.float32), res


def kernel(**inputs):
    out, _ = run(inputs)
    return out


def _jit_single_core(nc):
    """Build a single-device jitted callable for nc (same program as SPMD)."""
    import jax
    from concourse import bass2jax
    from concourse import mybir

    bass2jax.install_neuronx_cc_hook()
    in_names, out_names, out_avals, zero_outs = [], [], [], []
    partition_name = nc.partition_id_tensor.name if nc.partition_id_tensor else None
    for alloc in nc.m.functions[0].allocations:
        if not isinstance(alloc, mybir.MemoryLocationSet):
            continue
        name = alloc.memorylocations[0].name
        if alloc.kind == "ExternalInput":
            if name != partition_name:
                in_names.append(name)
        elif alloc.kind == "ExternalOutput":
            out_names.append(name)
            shape = tuple(alloc.tensor_shape)
            dtype = mybir.dt.np(alloc.dtype)
            out_avals.append(jax.core.ShapedArray(shape, dtype))
            zero_outs.append(np.zeros(shape, dtype))
    n_params = len(in_names)
    all_names = in_names + out_names + ([partition_name] if partition_name else [])
    donate = tuple(range(n_params, n_params + len(out_names)))

    def _body(*args):
        operands = list(args)
        if partition_name is not None:
            operands.append(bass2jax.partition_id_tensor())
        outs = bass2jax._bass_exec_p.bind(
            *operands,
            out_avals=tuple(out_avals),
            in_names=tuple(all_names),
            out_names=tuple(out_names),
            lowering_input_output_aliases=(),
            sim_require_finite=True,
            sim_require_nnan=True,
            nc=nc,
        )
        return tuple(outs)

    jfn = jax.jit(_body, donate_argnums=donate, keep_unused=True)
    return jfn, in_names, zero_outs


def bench_marginal(inputs, iters=24, reps=2):
    """Per-execution device time via async dispatch pipelining."""
    import time

    import jax

    x0_lbd, x0t2, ebt, S, wo, w1, w2 = _host_prep(inputs)
    if "nc" not in _NC_CACHE:
        _NC_CACHE["nc"] = build_nc()
    nc = _NC_CACHE["nc"]
    in_map = dict(
        x0b=np.ascontiguousarray(x0_lbd[:, :B_LOC, :]),
        x0t2=np.ascontiguousarray(x0t2[:, :B_LOC // 2, :]),
        ebt=np.ascontiguousarray(ebt[:, :B_LOC, :, :]),
        s=S, wo=wo, w1=w1, w2=w2,
    )
    jfn, in_names, zero_outs = _jit_single_core(nc)
    dev = jax.devices()[0]
    ins_dev = [jax.device_put(np.asarray(in_map[n]), dev) for n in in_names]
    n_zsets = (iters + 2) * reps + 4
    zsets = [
        [jax.device_put(z.copy(), dev) for z in zero_outs] for _ in range(n_zsets)
    ]
    jax.block_until_ready(zsets)
    jax.block_until_ready(ins_dev)
    state = {"zi": 0}

    def run_m(m):
        outs = []
        t0 = time.perf_counter()
        for _ in range(m):
            outs.append(jfn(*ins_dev, *zsets[state["zi"]]))
            state["zi"] += 1
        jax.block_until_ready(outs)
        return time.perf_counter() - t0

    run_m(1)  # warm (compiles)
    t1s, tns = [], []
    for _ in range(reps):
        t1s.append(run_m(1))
        tns.append(run_m(iters))
    marginal_ns = (min(tns) - min(t1s)) / (iters - 1) * 1e9
    return dict(
        est_exec_ns=marginal_ns,
        t1_ns=min(t1s) * 1e9,
        tn_ns=min(tns) * 1e9,
        t1s=t1s,
        tns=tns,
        iters=iters,
    )


if __name__ == "__main__":
    print("kernel module ok")
